# revision 21
# baseline (speedup 1.0000x reference)
"""Trainium2 Bass kernel for causal MLA self-attention.

Problem: B=2, T=2048, C=2048, H=16 heads, Dh=128, latent Dl=64.
  q = rope(x @ wq); k_lat = rope(x @ wk_lat); v_lat = x @ wv_lat
  k_h = k_lat @ k_expand[h]; v_h = v_lat @ v_expand[h]
  y = causal_softmax(q k^T / sqrt(Dh)) v;  out = y @ proj_w

Sharding: 8 cores = 2 batches x 4 head-groups (4 heads each).  Each core
computes a full (T, C) partial of the output projection restricted to its
heads; the host sums the 4 partials per batch.

Device algorithm (per core) uses the MLA absorption trick so attention
contracts over Dl=64 and only the tiny latent K/V is kept per core:
  qt_h = rope(q_h) @ k_expand[h]^T          (T, 64)
  s^T  = k_lat_rope @ qt_h^T                (Tk, Tq) tiles, exp on ScalarE
  yu^T = [v_lat | 1]^T @ exp(s^T)           (65, Tq): row 64 = softmax denom
  out += (yu/denom)^T_pair @ pv_pair        pv_h = v_expand[h] @ proj_w[h]
The v_expand/proj_w product is folded on the host, so the output
projection contracts 2 stacked heads' yu (2x64=128 partitions) against
pv pairs — half the matmul rows of the unfused expand+project.
All tensors are kept "transposed" (feature dim on partitions) so every
matmul contracts along partitions; softmax needs no max-subtraction
(scores are O(5)) and the denominator is a fused ones-column.
Attention internals (kk, qtil, exp-weights, vaug) and the streamed I/O
(x, wq, wkv, rope tables, output partials) are bf16; accumulation fp32.
"""

import os
import sys

import numpy as np

if not any(os.path.isdir(os.path.join(p, "concourse")) for p in sys.path if p):
    sys.path.insert(0, "/opt/trn_rl_repo")

import concourse.bass as bass  # noqa: E402
import concourse.mybir as mybir  # noqa: E402
import concourse.tile as tile  # noqa: E402
from concourse import bacc  # noqa: E402
from concourse.bass_utils import run_bass_kernel_spmd  # noqa: E402

B, T, C, H, Dh, Dl = 2, 2048, 2048, 16, 128, 64
HPC = 4  # heads per core
NCORES = 8
F32 = mybir.dt.float32
F32R = mybir.dt.float32r
BF16 = mybir.dt.bfloat16
SCALE = 1.0 / float(np.sqrt(Dh))

TJ = 512          # Tq chunk (matmul moving-dim)
NJ = T // TJ      # 4
NK = C // 128     # 16 contraction chunks over C
NTK = T // 128    # 16 Tk chunks


def build_nc():
    nc = bacc.Bacc(None, target_bir_lowering=False, debug=False)

    xT = nc.dram_tensor("xT", [C, T], F32R, kind="ExternalInput")
    wq = nc.dram_tensor("wq", [C, HPC * Dh], F32R, kind="ExternalInput")
    wkv = nc.dram_tensor("wkv", [C, 2 * Dl], F32R, kind="ExternalInput")
    eT = nc.dram_tensor("eT", [Dh, HPC * Dl], F32R, kind="ExternalInput")
    eT2 = nc.dram_tensor("eT2", [Dh, HPC * Dl], F32R, kind="ExternalInput")
    pv = nc.dram_tensor("pv", [2 * Dl, 2, C], F32R, kind="ExternalInput")
    cosq = nc.dram_tensor("cosq", [Dh, T], BF16, kind="ExternalInput")
    sinq = nc.dram_tensor("sinq", [Dh, T], BF16, kind="ExternalInput")
    cosk = nc.dram_tensor("cosk", [Dl, T], BF16, kind="ExternalInput")
    sink = nc.dram_tensor("sink", [Dl, T], BF16, kind="ExternalInput")
    sperm = nc.dram_tensor("sperm", [128, 128], F32R, kind="ExternalInput")
    ident = nc.dram_tensor("ident", [128, 128], F32, kind="ExternalInput")
    maskt = nc.dram_tensor("maskt", [128, 4, TJ], BF16, kind="ExternalInput")
    onec = nc.dram_tensor("onec", [128, NTK, Dl], F32R, kind="ExternalInput")
    out = nc.dram_tensor("out", [T, C], BF16, kind="ExternalOutput")

    with tile.TileContext(nc) as tc, \
         nc.allow_low_precision(reason="bf16 attention pipeline"):
        consts = tc.alloc_tile_pool(name="consts", bufs=1)

        # cross-phase intermediates
        kk_sb = consts.tile([128, T], F32R, name="kk_sb")      # k_rope^T x2
        qtil_sb = consts.tile([128, 2, T], F32R, name="qtil_sb")
        vaug_sb = consts.tile([128, NTK, 2 * Dl], F32R, name="vaug_sb")
        pv_sb = consts.tile([128, 2, C], F32R, name="pv_sb")
        maskt_sb = consts.tile([128, 4, TJ], BF16, name="maskt_sb")

        # ---------------- phase 1: projections + RoPE + q-tilde ----------
        with tc.tile_pool(name="ps1", bufs=1, space="PSUM") as ps1, \
             tc.tile_pool(name="ph1", bufs=1) as ph1:
            wq_sb = ph1.tile([128, NK, HPC * Dh], F32R, name="wq_sb")
            wq_r = wq[:].rearrange("(ko p) m -> p ko m", p=128)
            wkv_sb = ph1.tile([128, NK, 2 * Dl], F32R, name="wkv_sb")
            wkv_r = wkv[:].rearrange("(ko p) m -> p ko m", p=128)
            sperm_sb = ph1.tile([128, 128], F32R, name="sperm_sb")
            ident_sb = ph1.tile([128, 128], F32, name="ident_sb")
            eT_sb = ph1.tile([Dh, HPC * Dl], F32R, name="eT_sb")
            eT2_sb = ph1.tile([Dh, HPC * Dl], F32R, name="eT2_sb")
            cosq_sb = ph1.tile([Dh, T], BF16, name="cosq_sb")
            sinq_sb = ph1.tile([Dh, T], BF16, name="sinq_sb")
            cosk_sb = ph1.tile([Dl, T], BF16, name="cosk_sb")
            sink_sb = ph1.tile([Dl, T], BF16, name="sink_sb")
            klat_sb = ph1.tile([Dl, T], F32R, name="klat_sb")
            vT_sb = ph1.tile([Dl, T], F32, name="vT_sb")
            for j in range(NJ):
                js = slice(j * TJ, (j + 1) * TJ)
                xts = []
                # j==0: interleave xt/wkv/wq chunk loads so the fused
                # kv+q k-loop below streams at DMA rate from cycle one.
                for k in range(NK):
                    xt = ph1.tile([128, TJ], F32R, name=f"xt{j}_{k}",
                                  tag="xt", bufs=20)
                    nc.sync.dma_start(xt, xT[k * 128:(k + 1) * 128, js])
                    xts.append(xt)
                    if j == 0:
                        nc.sync.dma_start(wkv_sb[:, k, :], wkv_r[:, k, :])
                        nc.sync.dma_start(wq_sb[:, k, :], wq_r[:, k, :])
                if j == 0:
                    nc.sync.dma_start(cosq_sb, cosq[:])
                    nc.sync.dma_start(sinq_sb, sinq[:])
                    nc.sync.dma_start(cosk_sb, cosk[:])
                    nc.sync.dma_start(sink_sb, sink[:])
                    nc.sync.dma_start(sperm_sb, sperm[:])
                    nc.sync.dma_start(eT_sb, eT[:])
                    nc.sync.dma_start(eT2_sb, eT2[:])
                    nc.sync.dma_start(ident_sb, ident[:])
                    nc.sync.dma_start(vaug_sb[:, :, Dl:2 * Dl], onec[:])
                    nc.sync.dma_start(maskt_sb, maskt[:])
                    nc.sync.dma_start(
                        pv_sb, pv[:].rearrange("p q n -> p q n"))

                # fused kv + q projections: one pass over the xt chunks
                kvps = ps1.tile([128, TJ], F32, name=f"kvps{j}", tag="kv",
                                bufs=2)
                qps = [ps1.tile([128, TJ], F32, name=f"qps{j}_{m}", tag="q",
                                bufs=4) for m in range(HPC)]
                for k in range(NK):
                    nc.tensor.matmul(kvps, wkv_sb[:, k, :], xts[k],
                                     start=(k == 0), stop=(k == NK - 1))
                    for m in range(HPC):
                        nc.tensor.matmul(
                            qps[m], wq_sb[:, k, m * 128:(m + 1) * 128],
                            xts[k], start=(k == 0), stop=(k == NK - 1))
                # q heads: fused RoPE/absorb: qt = E (q*cos) + (E S)(q*sin')
                for m in range(HPC):
                    u1 = ph1.tile([128, TJ], F32R, name=f"u1_{j}_{m}",
                                  tag="u1", bufs=2)
                    nc.vector.tensor_mul(u1, qps[m], cosq_sb[:, js])
                    u2 = ph1.tile([128, TJ], F32R, name=f"u2_{j}_{m}",
                                  tag="u2", bufs=2)
                    nc.vector.tensor_mul(u2, qps[m], sinq_sb[:, js])
                    p, half = divmod(m, 2)
                    qtp = ps1.tile([64, TJ], F32, name=f"qtp{j}_{m}",
                                   tag="qt", bufs=1)
                    msl = slice(m * Dl, (m + 1) * Dl)
                    nc.tensor.matmul(qtp, eT_sb[:, msl], u1,
                                     start=True, stop=False)
                    nc.tensor.matmul(qtp, eT2_sb[:, msl], u2,
                                     start=False, stop=True)
                    nc.scalar.copy(
                        qtil_sb[half * Dl:(half + 1) * Dl, p, js], qtp)

                nc.vector.tensor_copy(klat_sb[:, js], kvps[0:Dl, :])
                nc.scalar.copy(vT_sb[:, js], kvps[Dl:128, :])

                # latent-k RoPE on the side engines
                ksps = ps1.tile([64, TJ], F32, name=f"ksps{j}", tag="swp",
                                bufs=1)
                nc.tensor.matmul(ksps, sperm_sb[0:Dl, 0:Dl],
                                 klat_sb[:, js], start=True, stop=True)
                tk1 = ph1.tile([Dl, TJ], F32, name=f"tk1_{j}", tag="tk1")
                nc.gpsimd.tensor_mul(tk1, klat_sb[:, js], cosk_sb[:, js])
                tk2 = ph1.tile([Dl, TJ], F32, name=f"tk2_{j}", tag="tk2")
                nc.vector.tensor_mul(tk2, ksps, sink_sb[:, js])
                nc.vector.tensor_add(kk_sb[0:Dl, js], tk1, tk2)
                nc.scalar.copy(kk_sb[Dl:128, js], kk_sb[0:Dl, js])

                # v_lat^T -> natural layout tiles [v | 1]
                for n in range(4 * j, 4 * j + 4):
                    vtp = ps1.tile([128, Dl], F32, name=f"vtp{n}", tag="swp",
                                   bufs=1)
                    nc.tensor.transpose(
                        vtp, vT_sb[:, n * 128:(n + 1) * 128],
                        ident_sb[0:Dl, 0:Dl])
                    nc.vector.tensor_copy(vaug_sb[:, n, 0:Dl], vtp)

        # ---------------- phase 2: attention + fused output proj ---------
        with tc.tile_pool(name="ps2", bufs=1, space="PSUM") as ps2, \
             tc.tile_pool(name="ph2", bufs=1) as ph2:
            for j in range(NJ):
                js = slice(j * TJ, (j + 1) * TJ)
                yup = ph2.tile([128, 2, TJ], F32R, name=f"yup{j}",
                               tag="yup", bufs=2)
                for p in range(2):
                    avt = ps2.tile([128, 2 * TJ], F32,
                                   name=f"av{j}_{p}", tag="av", bufs=1)
                    av = [avt[:, 0:TJ], avt[:, TJ:2 * TJ]]
                    nm = 4 * (j + 1)
                    for m in range(nm):
                        ms = slice(m * 128, (m + 1) * 128)
                        d = m - 4 * j if m >= 4 * j else -1
                        lo = 128 * d if d > 0 else 0
                        jsl = slice(j * TJ + lo, (j + 1) * TJ)
                        sps = ps2.tile([128, 2 * TJ], F32,
                                       name=f"sps{j}_{p}_{m}", tag="s",
                                       bufs=2)
                        nc.tensor.matmul(sps[:, lo:TJ], kk_sb[0:Dl, ms],
                                         qtil_sb[0:Dl, p, jsl],
                                         start=True, stop=True,
                                         tile_position=(0, 0))
                        nc.tensor.matmul(sps[:, TJ + lo:2 * TJ],
                                         kk_sb[Dl:128, ms],
                                         qtil_sb[Dl:128, p, jsl],
                                         start=True, stop=True,
                                         tile_position=(Dl, 0))
                        ex = ph2.tile([128, 2 * TJ], F32R,
                                      name=f"ex{j}_{p}_{m}", tag="ex", bufs=8)
                        if d >= 1:
                            # left columns fully masked: one strided exp over
                            # both halves' valid ranges
                            exv = ex[:].rearrange("q (a b) -> q a b", a=2)
                            spv = sps[:].rearrange("q (a b) -> q a b", a=2)
                            nc.scalar.activation(
                                exv[:, :, lo:TJ], spv[:, :, lo:TJ],
                                mybir.ActivationFunctionType.Exp, scale=SCALE)
                        else:
                            nc.scalar.activation(
                                ex, sps, mybir.ActivationFunctionType.Exp,
                                scale=SCALE)
                        if d >= 0:
                            # mask only the 128-wide staircase band
                            band = slice(lo, lo + 128)
                            nc.gpsimd.tensor_mul(
                                ex[:, band], ex[:, band],
                                maskt_sb[:, d, band])
                            band2 = slice(TJ + lo, TJ + lo + 128)
                            nc.vector.tensor_mul(
                                ex[:, band2], ex[:, band2],
                                maskt_sb[:, d, band])
                        for hf in range(2):
                            nc.tensor.matmul(
                                av[hf][:, lo:TJ], vaug_sb[:, m, :],
                                ex[:, hf * TJ + lo:(hf + 1) * TJ],
                                start=(m == 0), stop=(m == nm - 1))
                    # softmax denominators arrive pre-broadcast on partitions
                    # 64:128 (vaug ones block): reciprocal into sbuf, then
                    # divide while copying yu into the stacked pair tile
                    # (heads 2p / 2p+1 on partitions 0:64 / 64:128)
                    rba = ph2.tile([Dl, TJ], F32R, name=f"rba{j}_{p}",
                                   tag="rba", bufs=2)
                    nc.vector.reciprocal(rba, avt[Dl:128, 0:TJ])
                    rbb = ph2.tile([Dl, TJ], F32R, name=f"rbb{j}_{p}",
                                   tag="rbb", bufs=2)
                    nc.vector.reciprocal(rbb, avt[Dl:128, TJ:2 * TJ])
                    nc.vector.tensor_mul(yup[0:Dl, p, :], avt[0:Dl, 0:TJ],
                                         rba)
                    nc.vector.tensor_mul(yup[Dl:128, p, :],
                                         avt[0:Dl, TJ:2 * TJ], rbb)

                # fused output projection for this j's four Tq row-chunks:
                # contract stacked head-pairs (128) against pv pairs
                for mi in range(4 * j, 4 * j + 4):
                    msl = slice(mi * 128, (mi + 1) * 128)
                    lsl = slice((mi - 4 * j) * 128, (mi - 4 * j + 1) * 128)
                    for n in range(NJ):
                        pps = ps2.tile([128, TJ], F32, name=f"pps{mi}_{n}",
                                       tag="w", bufs=2)
                        for p in range(2):
                            nc.tensor.matmul(
                                pps, yup[:, p, lsl],
                                pv_sb[:, p, n * TJ:(n + 1) * TJ],
                                start=(p == 0), stop=(p == 1))
                        ot = ph2.tile([128, TJ], BF16, name=f"ot{mi}_{n}",
                                      tag="ot", bufs=6)
                        nc.vector.tensor_copy(ot, pps)
                        nc.sync.dma_start(out[msl, n * TJ:(n + 1) * TJ], ot)

        consts.release()

    nc.compile()
    return nc


def _rope_tables(t, d):
    inv = 1.0 / (10000.0 ** (np.arange(0, d, 2, dtype=np.float64) / d))
    ang = np.arange(t, dtype=np.float64)[:, None] * inv[None, :]  # (t, d/2)
    cos = np.cos(ang).T  # (d/2, t)
    sin = np.sin(ang).T
    cosf = np.empty((d, t), np.float32)
    sinf = np.empty((d, t), np.float32)
    cosf[0::2] = cos
    cosf[1::2] = cos
    sinf[0::2] = -sin
    sinf[1::2] = sin
    return cosf, sinf


def _host_inputs(x, wq, wk_lat, wv_lat, k_expand, v_expand, proj_w):
    import ml_dtypes
    bf = ml_dtypes.bfloat16

    cosq, sinq = _rope_tables(T, Dh)
    sinq = np.ascontiguousarray(sinq[np.arange(Dh) ^ 1, :])  # row-pair swap
    cosk, sink = _rope_tables(T, Dl)
    sperm = np.zeros((128, 128), np.float32)
    idx = np.arange(128)
    sperm[idx, idx ^ 1] = 1.0
    ident = np.eye(128, dtype=np.float32)
    tkr = np.arange(128)[:, None]
    tqr = np.arange(TJ)[None, :]
    maskt = np.stack(
        [(tkr + 128 * d <= tqr).astype(bf) for d in range(4)], axis=1)
    wkv = np.ascontiguousarray(np.concatenate([wk_lat, wv_lat], axis=1))


    xTs = [np.ascontiguousarray(x[b].T) for b in range(B)]
    in_maps = []
    for core in range(NCORES):
        b, g = divmod(core, 4)
        heads = range(4 * g, 4 * g + 4)
        eTc = np.ascontiguousarray(
            np.concatenate([k_expand[h].T for h in heads], axis=1))
        eT2c = np.ascontiguousarray(eTc[idx ^ 1, :])
        # pv pairs: pv[0:64, p] = v_expand[4g+2p] @ proj_w rows of head 4g+2p
        pvc = np.empty((128, 2, C), np.float32)
        for p in range(2):
            for hf in range(2):
                h = 4 * g + 2 * p + hf
                pvc[hf * Dl:(hf + 1) * Dl, p, :] = (
                    v_expand[h].astype(np.float64)
                    @ proj_w[h * Dh:(h + 1) * Dh, :].astype(np.float64)
                ).astype(np.float32)
        in_maps.append({
            "xT": xTs[b],
            "wq": np.ascontiguousarray(wq[:, g * 512:(g + 1) * 512]),
            "wkv": wkv,
            "eT": eTc, "eT2": eT2c,
            "pv": pvc,
            "cosq": cosq.astype(bf), "sinq": sinq.astype(bf),
            "cosk": cosk.astype(bf), "sink": sink.astype(bf),
            "sperm": sperm, "ident": ident, "maskt": maskt,
            "onec": np.ones((128, NTK, Dl), np.float32),
        })
    return in_maps


_NC_CACHE = {}


def run(inputs, trace=False, **kw):
    """Run on all 8 cores; returns (output, BassKernelResults)."""
    if "nc" not in _NC_CACHE:
        _NC_CACHE["nc"] = build_nc()
    nc = _NC_CACHE["nc"]
    in_maps = _host_inputs(**inputs)
    res = run_bass_kernel_spmd(
        nc, in_maps, core_ids=list(range(NCORES)), trace=trace, **kw)
    out = np.zeros((B, T, C), np.float32)
    for core in range(NCORES):
        out[core // 4] += res.results[core]["out"].astype(np.float32)
    return out, res


def kernel(**inputs):
    out, _ = run(inputs)
    return out


# revision 26
# speedup vs baseline: 1.0833x; 1.0833x over previous
"""Trainium2 Bass kernel for causal MLA self-attention.

Problem: B=2, T=2048, C=2048, H=16 heads, Dh=128, latent Dl=64.
  q = rope(x @ wq); k_lat = rope(x @ wk_lat); v_lat = x @ wv_lat
  k_h = k_lat @ k_expand[h]; v_h = v_lat @ v_expand[h]
  y = causal_softmax(q k^T / sqrt(Dh)) v;  out = y @ proj_w

Sharding: 8 cores = 2 batches x 4 head-groups (4 heads each).  Each core
computes a full (T, C) partial of the output projection restricted to its
heads; the host sums the 4 partials per batch.

Device algorithm (per core) uses the MLA absorption trick so attention
contracts over Dl=64 and only the tiny latent K/V is kept per core:
  qt_h = rope(q_h) @ k_expand[h]^T          (T, 64)
  s^T  = k_lat_rope @ qt_h^T                (Tk, Tq) tiles, exp on ScalarE
  yu^T = [v_lat | 1]^T @ exp(s^T)           (65, Tq): row 64 = softmax denom
  out += (yu/denom)^T_pair @ pv_pair        pv_h = v_expand[h] @ proj_w[h]
The v_expand/proj_w product is folded on the host, so the output
projection contracts 2 stacked heads' yu (2x64=128 partitions) against
pv pairs — half the matmul rows of the unfused expand+project.
All tensors are kept "transposed" (feature dim on partitions) so every
matmul contracts along partitions; softmax needs no max-subtraction
(scores are O(5)) and the denominator is a fused ones-column.
Attention internals (kk, qtil, exp-weights, vaug) and the streamed I/O
(x, wq, wkv, rope tables, output partials) are bf16; accumulation fp32.
"""

import os
import sys

import numpy as np

if not any(os.path.isdir(os.path.join(p, "concourse")) for p in sys.path if p):
    sys.path.insert(0, "/opt/trn_rl_repo")

import concourse.bass as bass  # noqa: E402
import concourse.mybir as mybir  # noqa: E402
import concourse.tile as tile  # noqa: E402
from concourse import bacc  # noqa: E402
from concourse.bass_utils import run_bass_kernel_spmd  # noqa: E402

B, T, C, H, Dh, Dl = 2, 2048, 2048, 16, 128, 64
HPC = 4  # heads per core
NCORES = 8
F32 = mybir.dt.float32
F32R = mybir.dt.float32r
BF16 = mybir.dt.bfloat16
SCALE = 1.0 / float(np.sqrt(Dh))

TJ = 512          # Tq chunk (matmul moving-dim)
NJ = T // TJ      # 4
NK = C // 128     # 16 contraction chunks over C
NTK = T // 128    # 16 Tk chunks


def build_nc():
    nc = bacc.Bacc(None, target_bir_lowering=False, debug=False)

    xT = nc.dram_tensor("xT", [C, T], F32R, kind="ExternalInput")
    wq = nc.dram_tensor("wq", [C, HPC * Dh], F32R, kind="ExternalInput")
    wkv = nc.dram_tensor("wkv", [C, 2 * Dl], F32R, kind="ExternalInput")
    eT = nc.dram_tensor("eT", [Dh, HPC * Dl], F32R, kind="ExternalInput")
    eT2 = nc.dram_tensor("eT2", [Dh, HPC * Dl], F32R, kind="ExternalInput")
    pv = nc.dram_tensor("pv", [2 * Dl, 2, C], F32R, kind="ExternalInput")
    cosq = nc.dram_tensor("cosq", [Dh, T], BF16, kind="ExternalInput")
    sinq = nc.dram_tensor("sinq", [Dh, T], BF16, kind="ExternalInput")
    cosk = nc.dram_tensor("cosk", [Dl, T], BF16, kind="ExternalInput")
    sink = nc.dram_tensor("sink", [Dl, T], BF16, kind="ExternalInput")
    sperm = nc.dram_tensor("sperm", [128, 128], F32R, kind="ExternalInput")
    ident = nc.dram_tensor("ident", [128, 128], F32, kind="ExternalInput")
    maskt = nc.dram_tensor("maskt", [128, 4, TJ], BF16, kind="ExternalInput")
    onec = nc.dram_tensor("onec", [128, NTK, Dl], F32R, kind="ExternalInput")
    out = nc.dram_tensor("out", [T, C], BF16, kind="ExternalOutput")

    with tile.TileContext(nc) as tc, \
         nc.allow_low_precision(reason="bf16 attention pipeline"):
        consts = tc.alloc_tile_pool(name="consts", bufs=1)

        # cross-phase intermediates
        kk_sb = consts.tile([128, T], F32R, name="kk_sb")      # k_rope^T x2
        qtil_sb = consts.tile([128, 2, T], F32R, name="qtil_sb")
        vaug_sb = consts.tile([128, NTK, 2 * Dl], F32R, name="vaug_sb")
        pv_sb = consts.tile([128, 2, C], F32R, name="pv_sb")
        maskt_sb = consts.tile([128, 4, TJ], BF16, name="maskt_sb")

        # ---------------- phase 1: projections + RoPE + q-tilde ----------
        with tc.tile_pool(name="ps1", bufs=1, space="PSUM") as ps1, \
             tc.tile_pool(name="ph1", bufs=1) as ph1:
            wq_sb = ph1.tile([128, NK, HPC * Dh], F32R, name="wq_sb")
            wq_r = wq[:].rearrange("(ko p) m -> p ko m", p=128)
            wkv_sb = ph1.tile([128, NK, 2 * Dl], F32R, name="wkv_sb")
            wkv_r = wkv[:].rearrange("(ko p) m -> p ko m", p=128)
            sperm_sb = ph1.tile([128, 128], F32R, name="sperm_sb")
            ident_sb = ph1.tile([128, 128], F32, name="ident_sb")
            eT_sb = ph1.tile([Dh, HPC * Dl], F32R, name="eT_sb")
            eT2_sb = ph1.tile([Dh, HPC * Dl], F32R, name="eT2_sb")
            cosq_sb = ph1.tile([Dh, T], BF16, name="cosq_sb")
            sinq_sb = ph1.tile([Dh, T], BF16, name="sinq_sb")
            cosk_sb = ph1.tile([Dl, T], BF16, name="cosk_sb")
            sink_sb = ph1.tile([Dl, T], BF16, name="sink_sb")
            klat_sb = ph1.tile([Dl, T], F32R, name="klat_sb")
            vT_sb = ph1.tile([Dl, T], F32, name="vT_sb")
            for j in range(NJ):
                js = slice(j * TJ, (j + 1) * TJ)
                xts = []
                # j==0: interleave xt/wkv/wq chunk loads so the fused
                # kv+q k-loop below streams at DMA rate from cycle one.
                for k in range(NK):
                    xt = ph1.tile([128, TJ], F32R, name=f"xt{j}_{k}",
                                  tag="xt", bufs=20)
                    nc.sync.dma_start(xt, xT[k * 128:(k + 1) * 128, js])
                    xts.append(xt)
                    if j == 0:
                        nc.sync.dma_start(wkv_sb[:, k, :], wkv_r[:, k, :])
                        nc.sync.dma_start(wq_sb[:, k, :], wq_r[:, k, :])
                if j == 0:
                    nc.sync.dma_start(cosq_sb, cosq[:])
                    nc.sync.dma_start(sinq_sb, sinq[:])
                    nc.sync.dma_start(cosk_sb, cosk[:])
                    nc.sync.dma_start(sink_sb, sink[:])
                    nc.sync.dma_start(sperm_sb, sperm[:])
                    nc.sync.dma_start(eT_sb, eT[:])
                    nc.sync.dma_start(eT2_sb, eT2[:])
                    nc.sync.dma_start(ident_sb, ident[:])
                if j == 2:
                    # phase-2 constants: late so they don't delay xt streams
                    nc.sync.dma_start(vaug_sb[:, :, Dl:2 * Dl], onec[:])
                    nc.sync.dma_start(maskt_sb, maskt[:])
                    nc.sync.dma_start(
                        pv_sb, pv[:].rearrange("p q n -> p q n"))

                # kv projection first; per-head q projections follow so each
                # head's RoPE/absorb overlaps the next head's matmuls
                kvps = ps1.tile([128, TJ], F32, name=f"kvps{j}", tag="kv",
                                bufs=2)
                for k in range(NK):
                    nc.tensor.matmul(kvps, wkv_sb[:, k, :], xts[k],
                                     start=(k == 0), stop=(k == NK - 1))
                nc.vector.tensor_copy(klat_sb[:, js], kvps[0:Dl, :])
                nc.scalar.copy(vT_sb[:, js], kvps[Dl:128, :])
                ksps = ps1.tile([64, TJ], F32, name=f"ksps{j}", tag="swp",
                                bufs=2)
                for m in range(HPC):
                    qp = ps1.tile([128, TJ], F32, name=f"qps{j}_{m}", tag="q",
                                  bufs=3)
                    for k in range(NK):
                        nc.tensor.matmul(
                            qp, wq_sb[:, k, m * 128:(m + 1) * 128],
                            xts[k], start=(k == 0), stop=(k == NK - 1))
                    if m == 0:
                        # PE filler while DVE computes u1/u2 for head 0
                        nc.tensor.matmul(ksps, sperm_sb[0:Dl, 0:Dl],
                                         klat_sb[:, js], start=True, stop=True)
                        for n in range(4 * j, 4 * j + 4):
                            vtp = ps1.tile([128, Dl], F32, name=f"vtp{n}",
                                           tag="swp", bufs=2)
                            nc.tensor.transpose(
                                vtp, vT_sb[:, n * 128:(n + 1) * 128],
                                ident_sb[0:Dl, 0:Dl])
                            nc.vector.tensor_copy(vaug_sb[:, n, 0:Dl], vtp)
                    u1 = ph1.tile([128, TJ], F32R, name=f"u1_{j}_{m}",
                                  tag="u1", bufs=2)
                    nc.vector.tensor_mul(u1, qp, cosq_sb[:, js])
                    u2 = ph1.tile([128, TJ], F32R, name=f"u2_{j}_{m}",
                                  tag="u2", bufs=2)
                    nc.vector.tensor_mul(u2, qp, sinq_sb[:, js])
                    p, half = divmod(m, 2)
                    qtp = ps1.tile([64, TJ], F32, name=f"qtp{j}_{m}",
                                   tag="qt", bufs=1)
                    msl = slice(m * Dl, (m + 1) * Dl)
                    nc.tensor.matmul(qtp, eT_sb[:, msl], u1,
                                     start=True, stop=False)
                    nc.tensor.matmul(qtp, eT2_sb[:, msl], u2,
                                     start=False, stop=True)
                    nc.scalar.copy(
                        qtil_sb[half * Dl:(half + 1) * Dl, p, js], qtp)

                # latent-k RoPE + vaug assembly on the side engines
                tk1 = ph1.tile([Dl, TJ], F32, name=f"tk1_{j}", tag="tk1")
                nc.gpsimd.tensor_mul(tk1, klat_sb[:, js], cosk_sb[:, js])
                tk2 = ph1.tile([Dl, TJ], F32, name=f"tk2_{j}", tag="tk2")
                nc.vector.tensor_mul(tk2, ksps, sink_sb[:, js])
                nc.vector.tensor_add(kk_sb[0:Dl, js], tk1, tk2)
                nc.scalar.copy(kk_sb[Dl:128, js], kk_sb[0:Dl, js])

        # ---------------- phase 2: attention + fused output proj ---------
        with tc.tile_pool(name="ps2", bufs=1, space="PSUM") as ps2, \
             tc.tile_pool(name="ph2", bufs=1) as ph2:
            for j in range(NJ):
                js = slice(j * TJ, (j + 1) * TJ)
                yup = ph2.tile([128, 2, TJ], F32R, name=f"yup{j}",
                               tag="yup", bufs=2)
                for p in range(2):
                    avt = ps2.tile([128, 2 * TJ], F32,
                                   name=f"av{j}_{p}", tag="av", bufs=1)
                    av = [avt[:, 0:TJ], avt[:, TJ:2 * TJ]]
                    nm = 4 * (j + 1)
                    for m in range(nm):
                        ms = slice(m * 128, (m + 1) * 128)
                        d = m - 4 * j if m >= 4 * j else -1
                        lo = 128 * d if d > 0 else 0
                        jsl = slice(j * TJ + lo, (j + 1) * TJ)
                        ex = ph2.tile([128, 2 * TJ], F32R,
                                      name=f"ex{j}_{p}_{m}", tag="ex", bufs=8)
                        for hf in range(2):
                            sph = ps2.tile([128, TJ], F32,
                                           name=f"sps{j}_{p}_{m}_{hf}",
                                           tag="s", bufs=4)
                            nc.tensor.matmul(
                                sph[:, lo:TJ], kk_sb[hf * Dl:hf * Dl + Dl, ms],
                                qtil_sb[hf * Dl:hf * Dl + Dl, p, jsl],
                                start=True, stop=True,
                                tile_position=(hf * Dl, 0))
                            exh = ex[:, hf * TJ:(hf + 1) * TJ]
                            nc.scalar.activation(
                                exh[:, lo:TJ], sph[:, lo:TJ],
                                mybir.ActivationFunctionType.Exp, scale=SCALE)
                            if d >= 0:
                                # mask only the 128-wide staircase band
                                band = slice(lo, lo + 128)
                                eng = nc.gpsimd if hf == 0 else nc.vector
                                eng.tensor_mul(
                                    exh[:, band], exh[:, band],
                                    maskt_sb[:, d, band])
                            nc.tensor.matmul(
                                av[hf][:, lo:TJ], vaug_sb[:, m, :],
                                exh[:, lo:TJ],
                                start=(m == 0), stop=(m == nm - 1))
                    # softmax denominators arrive pre-broadcast on partitions
                    # 64:128 (vaug ones block): reciprocal into sbuf, then
                    # divide while copying yu into the stacked pair tile
                    # (heads 2p / 2p+1 on partitions 0:64 / 64:128)
                    rba = ph2.tile([Dl, TJ], F32R, name=f"rba{j}_{p}",
                                   tag="rba", bufs=2)
                    nc.vector.reciprocal(rba, avt[Dl:128, 0:TJ])
                    rbb = ph2.tile([Dl, TJ], F32R, name=f"rbb{j}_{p}",
                                   tag="rbb", bufs=2)
                    nc.vector.reciprocal(rbb, avt[Dl:128, TJ:2 * TJ])
                    nc.vector.tensor_mul(yup[0:Dl, p, :], avt[0:Dl, 0:TJ],
                                         rba)
                    nc.vector.tensor_mul(yup[Dl:128, p, :],
                                         avt[0:Dl, TJ:2 * TJ], rbb)

                # fused output projection for this j's four Tq row-chunks:
                # contract stacked head-pairs (128) against pv pairs
                for mi in range(4 * j, 4 * j + 4):
                    msl = slice(mi * 128, (mi + 1) * 128)
                    lsl = slice((mi - 4 * j) * 128, (mi - 4 * j + 1) * 128)
                    for n in range(NJ):
                        pps = ps2.tile([128, TJ], F32, name=f"pps{mi}_{n}",
                                       tag="w", bufs=2)
                        for p in range(2):
                            nc.tensor.matmul(
                                pps, yup[:, p, lsl],
                                pv_sb[:, p, n * TJ:(n + 1) * TJ],
                                start=(p == 0), stop=(p == 1))
                        ot = ph2.tile([128, TJ], BF16, name=f"ot{mi}_{n}",
                                      tag="ot", bufs=6)
                        nc.vector.tensor_copy(ot, pps)
                        nc.sync.dma_start(out[msl, n * TJ:(n + 1) * TJ], ot)

        consts.release()

    nc.compile()
    return nc


def _rope_tables(t, d):
    inv = 1.0 / (10000.0 ** (np.arange(0, d, 2, dtype=np.float64) / d))
    ang = np.arange(t, dtype=np.float64)[:, None] * inv[None, :]  # (t, d/2)
    cos = np.cos(ang).T  # (d/2, t)
    sin = np.sin(ang).T
    cosf = np.empty((d, t), np.float32)
    sinf = np.empty((d, t), np.float32)
    cosf[0::2] = cos
    cosf[1::2] = cos
    sinf[0::2] = -sin
    sinf[1::2] = sin
    return cosf, sinf


def _host_inputs(x, wq, wk_lat, wv_lat, k_expand, v_expand, proj_w):
    import ml_dtypes
    bf = ml_dtypes.bfloat16

    cosq, sinq = _rope_tables(T, Dh)
    sinq = np.ascontiguousarray(sinq[np.arange(Dh) ^ 1, :])  # row-pair swap
    cosk, sink = _rope_tables(T, Dl)
    sperm = np.zeros((128, 128), np.float32)
    idx = np.arange(128)
    sperm[idx, idx ^ 1] = 1.0
    ident = np.eye(128, dtype=np.float32)
    tkr = np.arange(128)[:, None]
    tqr = np.arange(TJ)[None, :]
    maskt = np.stack(
        [(tkr + 128 * d <= tqr).astype(bf) for d in range(4)], axis=1)
    wkv = np.ascontiguousarray(np.concatenate([wk_lat, wv_lat], axis=1))


    xTs = [np.ascontiguousarray(x[b].T) for b in range(B)]
    in_maps = []
    for core in range(NCORES):
        b, g = divmod(core, 4)
        heads = range(4 * g, 4 * g + 4)
        eTc = np.ascontiguousarray(
            np.concatenate([k_expand[h].T for h in heads], axis=1))
        eT2c = np.ascontiguousarray(eTc[idx ^ 1, :])
        # pv pairs: pv[0:64, p] = v_expand[4g+2p] @ proj_w rows of head 4g+2p
        pvc = np.empty((128, 2, C), np.float32)
        for p in range(2):
            for hf in range(2):
                h = 4 * g + 2 * p + hf
                pvc[hf * Dl:(hf + 1) * Dl, p, :] = (
                    v_expand[h].astype(np.float64)
                    @ proj_w[h * Dh:(h + 1) * Dh, :].astype(np.float64)
                ).astype(np.float32)
        in_maps.append({
            "xT": xTs[b],
            "wq": np.ascontiguousarray(wq[:, g * 512:(g + 1) * 512]),
            "wkv": wkv,
            "eT": eTc, "eT2": eT2c,
            "pv": pvc,
            "cosq": cosq.astype(bf), "sinq": sinq.astype(bf),
            "cosk": cosk.astype(bf), "sink": sink.astype(bf),
            "sperm": sperm, "ident": ident, "maskt": maskt,
            "onec": np.ones((128, NTK, Dl), np.float32),
        })
    return in_maps


_NC_CACHE = {}


def run(inputs, trace=False, **kw):
    """Run on all 8 cores; returns (output, BassKernelResults)."""
    if "nc" not in _NC_CACHE:
        _NC_CACHE["nc"] = build_nc()
    nc = _NC_CACHE["nc"]
    in_maps = _host_inputs(**inputs)
    res = run_bass_kernel_spmd(
        nc, in_maps, core_ids=list(range(NCORES)), trace=trace, **kw)
    out = np.zeros((B, T, C), np.float32)
    for core in range(NCORES):
        out[core // 4] += res.results[core]["out"].astype(np.float32)
    return out, res


def kernel(**inputs):
    out, _ = run(inputs)
    return out
